# revision 2
# baseline (speedup 1.0000x reference)
"""Trainium2 Bass kernel for nn_EMHA (strided sparse attention block).

Math (per batch b of 4):
  XR = Wr @ x[b] + br                       (512, 4096)
  H  = raw view of XR as (4096, 512)        [free reshape in flat space]
  q/k/v = per-64-col-block H @ W{q,k,v}.T   (same 64x64 W for all 8 head-blocks)
  The (B,N,M,HD)->(B,N/S,M,S,HD) raw reshape + einsums reduce exactly to:
  32 independent attention groups (r = n%4, m = head): rows n==r (mod 4),
  cols [64m,64m+64), each a (1024 x 1024) softmax attention.
  OutMat (4096,512) viewed as (512,4096); out[b] = We @ OutMat_view + be.

Sharding: 8 cores = (b in 0..4) x (head-group hg in 0..2, 4 heads each).
A core only needs x / produces out columns n' with (n'%512)//256 == hg
(8 interleaved 256-wide stripes) -> no inter-core communication.

v2 speedups over the f32r baseline:
  - all dense GEMMs (XR reduction, q/k/v projections, We expansion) run in
    bf16 (same PE rate as f32r at wide ap, but half the DMA/SBUF traffic and
    it unlocks the 1-cycle/row rate for the narrow V-projection matmuls);
  - the energy matmul runs in fp8e4 with MatmulPerfMode.DoubleRow (2 rows
    per cycle): the pair dimension carries an error-compensated double
    rounding (q0 = fp8(q), q1 = fp8(2q - q0)), so q0*k0 + q1*k1 = 2*q*k with
    first-order quantization error cancelled; the 1/2 folds into the
    softmax scale (1/64 instead of 1/32);
  - merged prologue DMAs (one descriptor per tensor / x-stripe, host-side
    pre-transposed layouts).

Emission is software-pipelined for the in-order engines: attention rounds
(E^T matmuls -> exp on ACT -> A@V) are the ACT-paced backbone; stage-1/2
chunks (x DMA, XR^T, QKV) and the final We matmul chunks are drained from a
gated filler queue into the rounds' PE slack.
"""

import numpy as np

EMBED, M, S, HD = 1024, 8, 4, 64
B, N = 4, 4096
NCORES = 8

_SCALE2 = 1.0 / 64.0  # 1/(2*sqrt(EMBED)): dither-pair halving folded in


def _build_nc(pack_e=True, repeat=1, loop_trips=0, filler_per_jb=3,
              av_jc=8, et_ic=2):
    import contextlib

    import concourse.tile as tile
    from concourse import bacc, mybir

    dt = mybir.dt
    f32 = dt.float32
    bf16 = dt.bfloat16
    f8 = dt.float8e4

    nc = bacc.Bacc(None, target_bir_lowering=False)

    xs = nc.dram_tensor("xs", [8, 128, 8, 256], bf16, kind="ExternalInput")
    wrt = nc.dram_tensor("wrt", [128, 8, 512], bf16, kind="ExternalInput")
    brb = nc.dram_tensor("brb", [128, 512], f32, kind="ExternalInput")
    bdqkv = nc.dram_tensor("bdqkv", [128, 3, 128], bf16, kind="ExternalInput")
    wet = nc.dram_tensor("wet", [128, 4, 1024], bf16, kind="ExternalInput")
    beb = nc.dram_tensor("beb", [128, 8], f32, kind="ExternalInput")
    out = nc.dram_tensor("out", [1024, 2048], f32, kind="ExternalOutput")

    with tile.TileContext(nc) as tc:
        with (
            tc.tile_pool(name="persist", bufs=1) as persist,
            tc.tile_pool(name="big", bufs=4) as bigpool,
            tc.tile_pool(name="htg", bufs=4) as htgp,
            tc.tile_pool(name="xin", bufs=2) as xin,
            tc.tile_pool(name="outp", bufs=3) as outp,
            tc.tile_pool(name="small", bufs=4) as small,
            tc.tile_pool(name="dtmp", bufs=2) as dtmp,
            tc.tile_pool(name="ps512", bufs=2, space="PSUM") as ps512,
            tc.tile_pool(name="pse", bufs=2, space="PSUM") as pse,
            tc.tile_pool(name="ps128", bufs=2, space="PSUM") as ps128,
        ):
            # ---- constants; x stripes 0/4 kicked off first (critical path) --
            x_first = {}
            for g in (0, 4):
                x_first[g] = xin.tile([128, 8, 256], bf16, tag="xin",
                                      name=f"xpro_{g}")
                nc.sync.dma_start(x_first[g][:], xs[g])
            wrt_sb = persist.tile([128, 8, 512], bf16, tag="wrt")
            nc.sync.dma_start(wrt_sb[:], wrt[:])
            brb_sb = persist.tile([128, 512], f32, tag="brb")
            nc.sync.dma_start(brb_sb[:], brb[:])
            bdqkv_sb = persist.tile([128, 3, 128], bf16, tag="bdqkv")
            nc.sync.dma_start(bdqkv_sb[:], bdqkv[:])
            beb_sb = persist.tile([128, 8], f32, tag="beb")
            nc.sync.dma_start(beb_sb[:], beb[:])
            wet_sb = persist.tile([128, 4, 1024], bf16, tag="wet")

            if loop_trips > 1:
                rep_ctxs = [tc.For_i(0, loop_trips, 1)]
            else:
                rep_ctxs = [contextlib.nullcontext(None) for _ in range(repeat)]

            for _rep, _ctx in enumerate(rep_ctxs):
              with _ctx:
                # [feat 128, g 8, dither 2, row 512] fp8e4
                qT = [persist.tile([128, 8, 2, 512], f8, tag=f"qT{p}",
                                   name=f"qT{_rep}_{p}") for p in range(2)]
                kT = [persist.tile([128, 8, 2, 512], f8, tag=f"kT{p}",
                                   name=f"kT{_rep}_{p}") for p in range(2)]
                # V_sb[p][sig, sb, grp*65 + c]; col 64 of each 65-block = ones
                V_sb = [persist.tile([128, 32, 130], bf16, tag=f"V{p}",
                                     name=f"V{_rep}_{p}") for p in range(2)]
                for p in range(2):
                    nc.vector.memset(V_sb[p][:, :, 64:65], 1.0)
                    nc.vector.memset(V_sb[p][:, :, 129:130], 1.0)
                OutMat = persist.tile([128, 4, 2048], bf16, tag="outmat",
                                      name=f"OutMat{_rep}")

                # ---------- stage-1/2 chunk emitters (filler steps) ----------
                def g_block_steps(g, x_pre=None):
                    """Emit x-DMA now; return 10 filler closures (5 per pair)."""
                    if x_pre is not None:
                        x_sb = x_pre
                    else:
                        x_sb = xin.tile([128, 8, 256], bf16, tag="xin",
                                        name=f"x{_rep}_{g}")
                        nc.sync.dma_start(x_sb[:], xs[g])
                    steps = []
                    for p in range(2):
                        ht = [None]
                        acc = [None]

                        def s1(p=p, ht=ht, acc=acc, g=g):
                            ht[0] = htgp.tile([128, 512], bf16, tag="htg",
                                              name=f"ht{_rep}_{g}_{p}")
                            acc[0] = ps512.tile([128, 512], f32, tag="ps512",
                                                name=f"xacc{_rep}_{g}_{p}")
                            for kc in range(4):
                                nc.tensor.matmul(
                                    acc[0][:],
                                    x_sb[:, kc, p * 128:(p + 1) * 128],
                                    wrt_sb[:, kc, :],
                                    start=(kc == 0), stop=False)

                        def s2(p=p, ht=ht, acc=acc):
                            for kc in range(4, 8):
                                nc.tensor.matmul(
                                    acc[0][:],
                                    x_sb[:, kc, p * 128:(p + 1) * 128],
                                    wrt_sb[:, kc, :],
                                    start=False, stop=(kc == 7))
                            nc.vector.tensor_add(ht[0][:], acc[0][:], brb_sb[:])

                        def qk_step(which, dst, p, ht, g):
                            pq = ps512.tile([128, 512], f32, tag="ps512",
                                            name=f"p{which}{_rep}_{g}_{p}")
                            nc.tensor.matmul(
                                pq[:], bdqkv_sb[:, 0 if which == "q" else 1, :],
                                ht[0][:], start=True, stop=True)
                            nc.vector.tensor_copy(out=dst[:, g, 0, :],
                                                  in_=pq[:])
                            dd = dtmp.tile([128, 512], bf16, tag="dtmp",
                                           name=f"d{which}{_rep}_{g}_{p}")
                            nc.vector.tensor_tensor(
                                dd[:], pq[:], dst[:, g, 0, :],
                                mybir.AluOpType.subtract)
                            nc.vector.tensor_tensor(
                                dst[:, g, 1, :], pq[:], dd[:],
                                mybir.AluOpType.add)

                        def s3(p=p, ht=ht, g=g):
                            qk_step("q", qT[p], p, ht, g)

                        def s4(p=p, ht=ht, g=g):
                            qk_step("k", kT[p], p, ht, g)

                        def s5(p=p, ht=ht, g=g):
                            for sub in range(4):
                                sb = g * 4 + sub
                                pv = ps128.tile([128, 130], f32, tag="ps128",
                                                name=f"pv{_rep}_{g}_{p}_{sub}")
                                nc.tensor.matmul(
                                    pv[:, 0:128],
                                    ht[0][:, sub * 128:(sub + 1) * 128],
                                    bdqkv_sb[:, 2, :],
                                    start=True, stop=True)
                                nc.vector.tensor_copy(
                                    out=V_sb[p][:, sb, :].rearrange(
                                        "q (gg c) -> q gg c", gg=2)[:, :, 0:64],
                                    in_=pv[:, 0:128].rearrange(
                                        "q (gg c) -> q gg c", gg=2))

                        steps += [s1, s2, s3, s4, s5]
                    return steps

                def we_u_steps(u):
                    """Final matmul for output columns [u*256, (u+1)*256)."""
                    steps = []
                    for ob in range(8):
                        def s(ob=ob, u=u):
                            pf = ps512.tile([128, 256], f32, tag="ps512",
                                            name=f"pf{_rep}_{u}_{ob}")
                            for cc in range(4):
                                nc.tensor.matmul(
                                    pf[:],
                                    wet_sb[:, cc, ob * 128:(ob + 1) * 128],
                                    OutMat[:, cc, u * 256:(u + 1) * 256],
                                    start=(cc == 0), stop=(cc == 3))
                            ot = outp.tile([128, 256], f32, tag="outp",
                                           name=f"ot{_rep}_{u}_{ob}")
                            nc.vector.tensor_scalar_add(
                                out=ot[:], in0=pf[:], scalar1=beb_sb[:, ob:ob + 1])
                            nc.sync.dma_start(
                                out[ob * 128:(ob + 1) * 128,
                                    u * 256:(u + 1) * 256], ot[:])
                        steps.append(s)
                    return steps

                # gated filler queue: (gate_t, closure); consumable when the
                # current round index t >= gate_t
                filler = []

                def drain_filler(t, budget=None):
                    n = 0
                    while filler and filler[0][0] <= t and (
                            budget is None or n < budget):
                        filler.pop(0)[1]()
                        n += 1

                # ---------- attention round emitters ----------
                def emit_av_ib(t, ee, ib):
                    rr, p = t // 2, t % 2
                    po = ps128.tile([128, 130], f32, tag="ps128",
                                    name=f"po{_rep}_{t}_{ib}")
                    for grp in range(2):
                        for jc in range(av_jc):
                            sbj = 4 * rr + jc if jc < 4 else 4 * (rr + 4) + (jc - 4)
                            nc.tensor.matmul(
                                po[:, grp * 65:grp * 65 + 65],
                                ee[grp][:, jc, ib * 128:ib * 128 + 128],
                                V_sb[p][:, sbj, grp * 65:grp * 65 + 65],
                                start=(jc == 0), stop=(jc == av_jc - 1))
                    pov = po[:].rearrange("q (gg c) -> q gg c", gg=2)
                    rec = small.tile([128, 2], f32, tag="rec",
                                     name=f"rec{_rep}_{t}_{ib}")
                    nc.vector.reciprocal(out=rec[:], in_=pov[:, :, 64])
                    u = rr if ib < 4 else rr + 4
                    col = u * 256 + p * 128
                    nc.vector.tensor_tensor(
                        OutMat[:, ib % 4, col:col + 128].rearrange(
                            "q (gg c) -> q gg c", gg=2),
                        pov[:, :, 0:64],
                        rec[:, :, None].to_broadcast((128, 2, 64)),
                        mybir.AluOpType.mult)

                def emit_round(t, prev_ee):
                    """E^T + exp for round t, with AV of t-1 and filler woven in."""
                    rr, p = t // 2, t % 2
                    ee = [bigpool.tile([128, 8, 1024], bf16, tag="big",
                                       name=f"ee{_rep}_{t}_{g_}")
                          for g_ in range(2)]
                    for jb in range(8):
                        gj = rr if jb < 4 else rr + 4
                        cj = (jb % 4) * 128
                        pe_t = [pse.tile([128, 1024], f32, tag="pse",
                                         name=f"pe{_rep}_{t}_{jb}_{g_}")
                                for g_ in range(2)]
                        for grp in range(2):
                            rows = slice(grp * 64, grp * 64 + 64)
                            for ic in range(et_ic):
                                gi = rr if ic == 0 else rr + 4
                                nc.tensor.matmul(
                                    pe_t[grp][:, ic * 512:(ic + 1) * 512],
                                    kT[p][rows, gj, :, cj:cj + 128],
                                    qT[p][rows, gi, :, :],
                                    start=True, stop=True,
                                    perf_mode=mybir.MatmulPerfMode.DoubleRow,
                                    tile_position=(grp * 64, 0))
                        for grp in range(2):
                            nc.scalar.activation(
                                out=ee[grp][:, jb, :],
                                in_=pe_t[grp][:],
                                func=mybir.ActivationFunctionType.Exp,
                                scale=_SCALE2)
                        if prev_ee is not None:
                            emit_av_ib(t - 1, prev_ee, jb)
                        drain_filler(t, budget=filler_per_jb)
                    return ee

                # ---------- prologue: stripes for round r=0 ----------
                deferred = []
                for g in (0, 4):
                    st = g_block_steps(g, x_pre=x_first[g])
                    for i, s in enumerate(st):
                        if i % 5 == 4:  # defer V-blocks (only needed by AV)
                            deferred.append((0, s))
                        else:
                            s()
                nc.sync.dma_start(wet_sb[:], wet[:])

                # queue remaining stripes + We chunks with gates; stripe pairs
                # {1,5}/{2,6}/{3,7} must be fully emitted before rounds t=2/4/6
                # respectively (enforced by the force-drain below); we_u(u) is
                # emittable once AV for its r is fully emitted (AV of round T
                # is woven into round T+1).
                filler.extend(deferred)
                for g in (1, 5):
                    for s in g_block_steps(g):
                        filler.append((0, s))
                for g in (2, 6):
                    for s in g_block_steps(g):
                        filler.append((1, s))
                for g in (3, 7):
                    for s in g_block_steps(g):
                        filler.append((3, s))
                we_gate = {0: 3, 4: 3, 1: 5, 5: 5, 2: 7, 6: 7}
                for u in (0, 4, 1, 5, 2, 6):
                    for s in we_u_steps(u):
                        filler.append((we_gate[u], s))

                prev = None
                for t in range(8):
                    # force-drain stripes required for this round's q/k reads
                    if t in (2, 4, 6):
                        drain_filler({2: 0, 4: 1, 6: 3}[t])
                    prev = emit_round(t, prev)
                drain_filler(7)
                for ib in range(4):
                    emit_av_ib(7, prev, ib)
                for s in we_u_steps(3):
                    s()
                for ib in range(4, 8):
                    emit_av_ib(7, prev, ib)
                for s in we_u_steps(7):
                    s()

    nc.finalize()
    return nc


def _prep_inputs(x, Wq, Wk, Wv, Wr, br, We, be):
    import ml_dtypes

    bfdt = ml_dtypes.bfloat16
    x = np.asarray(x, np.float32)
    Wr = np.asarray(Wr, np.float32)
    We = np.asarray(We, np.float32)

    wrt2 = np.empty((128, 8, 512), np.float32)  # [P, kc, c] = Wr[c, kc*128+P]
    for kc in range(8):
        wrt2[:, kc, :] = Wr[:, kc * 128:(kc + 1) * 128].T
    wet2 = np.empty((128, 4, 1024), np.float32)  # [P, cc, o] = We[o, cc*128+P]
    for cc in range(4):
        wet2[:, cc, :] = We[:, cc * 128:(cc + 1) * 128].T
    brb = np.ascontiguousarray(
        np.broadcast_to(np.asarray(br, np.float32)[None, :], (128, 512)))
    beb = np.ascontiguousarray(np.asarray(be, np.float32).reshape(8, 128).T)

    def bd(w):
        z = np.zeros((128, 128), np.float32)
        wt = np.asarray(w, np.float32).T
        z[:64, :64] = wt
        z[64:, 64:] = wt
        return z

    bdqkv = np.stack([bd(Wq), bd(Wk), bd(Wv)], axis=1)  # [128, 3, 128]
    shared = dict(wrt=wrt2.astype(bfdt), wet=wet2.astype(bfdt), brb=brb,
                  beb=beb, bdqkv=bdqkv.astype(bfdt))
    in_maps = []
    for core in range(NCORES):
        b, hg = core // 2, core % 2
        xv = x[b].reshape(1024, 8, 2, 256)[:, :, hg, :]  # [ch, g, cc]
        xsh = np.empty((8, 128, 8, 256), np.float32)     # [g, P, kc, cc]
        for kc in range(8):
            xsh[:, :, kc, :] = xv[kc * 128:(kc + 1) * 128].transpose(1, 0, 2)
        in_maps.append(dict(xs=xsh.astype(bfdt), **shared))
    return in_maps


def kernel(x, Wq, Wk, Wv, Wr, br, We, be, _trace=False, _pack_e=True):
    from concourse.bass_utils import run_bass_kernel_spmd

    nc = _build_nc(pack_e=_pack_e)
    in_maps = _prep_inputs(x, Wq, Wk, Wv, Wr, br, We, be)
    res = run_bass_kernel_spmd(nc, in_maps, core_ids=list(range(NCORES)),
                               trace=_trace)
    outa = np.zeros((B, EMBED, N), np.float32)
    for core in range(NCORES):
        b, hg = core // 2, core % 2
        oc = res.results[core]["out"]
        outa[b].reshape(1024, 8, 2, 256)[:, :, hg, :] = oc.reshape(1024, 8, 256)
    if _trace:
        kernel._last_results = res
    return outa


# revision 16
# speedup vs baseline: 1.1995x; 1.1995x over previous
"""Trainium2 Bass kernel for nn_EMHA (strided sparse attention block).

Math (per batch b of 4):
  XR = Wr @ x[b] + br                       (512, 4096)
  H  = raw view of XR as (4096, 512)        [free reshape in flat space]
  q/k/v = per-64-col-block H @ W{q,k,v}.T   (same 64x64 W for all 8 head-blocks)
  The (B,N,M,HD)->(B,N/S,M,S,HD) raw reshape + einsums reduce exactly to:
  32 independent attention groups (r = n%4, m = head): rows n==r (mod 4),
  cols [64m,64m+64), each a (1024 x 1024) softmax attention.
  OutMat (4096,512) viewed as (512,4096); out[b] = We @ OutMat_view + be.

Sharding: 8 cores = (b in 0..4) x (head-group hg in 0..2, 4 heads each).
A core only needs x / produces out columns n' with (n'%512)//256 == hg
(8 interleaved 256-wide stripes) -> no inter-core communication.

v2 speedups over the f32r baseline:
  - all dense GEMMs (XR reduction, q/k/v projections, We expansion) run in
    bf16 (same PE rate as f32r at wide ap, but half the DMA/SBUF traffic and
    it unlocks the 1-cycle/row rate for the narrow V-projection matmuls);
  - the energy matmul runs in fp8e4 with MatmulPerfMode.DoubleRow (2 rows
    per cycle): the pair dimension carries an error-compensated double
    rounding (q0 = fp8(q), q1 = fp8(2q - q0)), so q0*k0 + q1*k1 = 2*q*k with
    first-order quantization error cancelled; the 1/2 folds into the
    softmax scale (1/64 instead of 1/32);
  - merged prologue DMAs (one descriptor per tensor / x-stripe, host-side
    pre-transposed layouts).

Emission is software-pipelined for the in-order engines: attention rounds
(E^T matmuls -> exp on ACT -> A@V) are the ACT-paced backbone; stage-1/2
chunks (x DMA, XR^T, QKV) and the final We matmul chunks are drained from a
gated filler queue into the rounds' PE slack.
"""

import numpy as np

EMBED, M, S, HD = 1024, 8, 4, 64
B, N = 4, 4096
NCORES = 8

_SCALE2 = 1.0 / 32.0  # 1/sqrt(EMBED)


def _build_nc(pack_e=True, repeat=1, loop_trips=0, filler_per_jb=3,
              av_jc=8, et_ic=2):
    import contextlib

    import concourse.tile as tile
    from concourse import bacc, mybir

    dt = mybir.dt
    f32 = dt.float32
    bf16 = dt.bfloat16
    f8 = dt.float8e3

    nc = bacc.Bacc(None, target_bir_lowering=False)

    xs = nc.dram_tensor("xs", [8, 128, 8, 256], bf16, kind="ExternalInput")
    wrt = nc.dram_tensor("wrt", [128, 8, 512], bf16, kind="ExternalInput")
    brb = nc.dram_tensor("brb", [128, 512], f32, kind="ExternalInput")
    bdqkv = nc.dram_tensor("bdqkv", [128, 3, 128], bf16, kind="ExternalInput")
    wet = nc.dram_tensor("wet", [128, 4, 1024], bf16, kind="ExternalInput")
    beb = nc.dram_tensor("beb", [128, 8], f32, kind="ExternalInput")
    out = nc.dram_tensor("out", [1024, 2048], f32, kind="ExternalOutput")

    with tile.TileContext(nc) as tc:
        with (
            tc.tile_pool(name="persist", bufs=1) as persist,
            tc.tile_pool(name="big", bufs=4) as bigpool,
            tc.tile_pool(name="htg", bufs=4) as htgp,
            tc.tile_pool(name="xin", bufs=4) as xin,
            tc.tile_pool(name="outp", bufs=3) as outp,
            tc.tile_pool(name="small", bufs=4) as small,
            tc.tile_pool(name="dtmp", bufs=2) as dtmp,
            tc.tile_pool(name="ps512", bufs=2, space="PSUM") as ps512,
            tc.tile_pool(name="pse", bufs=2, space="PSUM") as pse,
            tc.tile_pool(name="ps128", bufs=2, space="PSUM") as ps128,
        ):
            # ---- constants; x stripes 0/4 kicked off first (critical path) --
            x_first = {}
            for g in (0, 4):
                x_first[g] = xin.tile([128, 8, 256], bf16, tag="xin",
                                      name=f"xpro_{g}")
                nc.sync.dma_start(x_first[g][:], xs[g])
            wrt_sb = persist.tile([128, 8, 512], bf16, tag="wrt")
            nc.sync.dma_start(wrt_sb[:], wrt[:])
            brb_sb = persist.tile([128, 512], f32, tag="brb")
            nc.sync.dma_start(brb_sb[:], brb[:])
            bdqkv_sb = persist.tile([128, 3, 128], bf16, tag="bdqkv")
            nc.sync.dma_start(bdqkv_sb[:], bdqkv[:])
            beb_sb = persist.tile([128, 8], f32, tag="beb")
            nc.sync.dma_start(beb_sb[:], beb[:])
            wet_sb = persist.tile([128, 4, 1024], bf16, tag="wet")

            if loop_trips > 1:
                rep_ctxs = [tc.For_i(0, loop_trips, 1)]
            else:
                rep_ctxs = [contextlib.nullcontext(None) for _ in range(repeat)]

            for _rep, _ctx in enumerate(rep_ctxs):
              with _ctx:
                # q/k in fp8e3 (E3M4: 4 mantissa bits, ~1.2% rms — fits the
                # error budget without dither pairs); DoublePixel mode
                # processes 2 output columns/cycle for fp8 operands.
                qT = [persist.tile([128, 8, 512], f8, tag=f"qT{p}",
                                   name=f"qT{_rep}_{p}") for p in range(2)]
                kT = [persist.tile([128, 8, 512], f8, tag=f"kT{p}",
                                   name=f"kT{_rep}_{p}") for p in range(2)]
                # V_sb[p][sig, sb, grp*65 + c]; col 64 of each 65-block = ones
                V_sb = [persist.tile([128, 32, 130], bf16, tag=f"V{p}",
                                     name=f"V{_rep}_{p}") for p in range(2)]
                for p in range(2):
                    nc.vector.memset(V_sb[p][:, :, 64:65], 1.0)
                    nc.vector.memset(V_sb[p][:, :, 129:130], 1.0)
                OutMat = persist.tile([128, 4, 2048], bf16, tag="outmat",
                                      name=f"OutMat{_rep}")

                # ---------- stage-1/2 chunk emitters (filler steps) ----------
                def g_block_steps(g, x_pre=None):
                    """Emit x-DMA now; return 10 filler closures (5 per pair)."""
                    if x_pre is not None:
                        x_sb = x_pre
                    else:
                        x_sb = xin.tile([128, 8, 256], bf16, tag="xin",
                                        name=f"x{_rep}_{g}")
                        nc.sync.dma_start(x_sb[:], xs[g])
                    steps = []
                    for p in range(2):
                        ht = [None]
                        acc = [None]

                        def s1(p=p, ht=ht, acc=acc, g=g):
                            ht[0] = htgp.tile([128, 512], bf16, tag="htg",
                                              name=f"ht{_rep}_{g}_{p}")
                            acc[0] = ps512.tile([128, 512], f32, tag="ps512",
                                                name=f"xacc{_rep}_{g}_{p}")
                            for kc in range(4):
                                nc.tensor.matmul(
                                    acc[0][:],
                                    x_sb[:, kc, p * 128:(p + 1) * 128],
                                    wrt_sb[:, kc, :],
                                    start=(kc == 0), stop=False)

                        def s2(p=p, ht=ht, acc=acc):
                            for kc in range(4, 8):
                                nc.tensor.matmul(
                                    acc[0][:],
                                    x_sb[:, kc, p * 128:(p + 1) * 128],
                                    wrt_sb[:, kc, :],
                                    start=False, stop=(kc == 7))
                            nc.vector.tensor_add(ht[0][:], acc[0][:], brb_sb[:])

                        def qk_step(which, dst, p, ht, g):
                            pq = ps512.tile([128, 512], f32, tag="ps512",
                                            name=f"p{which}{_rep}_{g}_{p}")
                            nc.tensor.matmul(
                                pq[:], bdqkv_sb[:, 0 if which == "q" else 1, :],
                                ht[0][:], start=True, stop=True)
                            nc.vector.tensor_copy(out=dst[:, g, :], in_=pq[:])

                        def s3(p=p, ht=ht, g=g):
                            qk_step("q", qT[p], p, ht, g)

                        def s4(p=p, ht=ht, g=g):
                            qk_step("k", kT[p], p, ht, g)

                        def s5(p=p, ht=ht, g=g):
                            for sub in range(4):
                                sb = g * 4 + sub
                                pv = ps128.tile([128, 130], f32, tag="ps128",
                                                name=f"pv{_rep}_{g}_{p}_{sub}")
                                nc.tensor.matmul(
                                    pv[:, 0:128],
                                    ht[0][:, sub * 128:(sub + 1) * 128],
                                    bdqkv_sb[:, 2, :],
                                    start=True, stop=True)
                                nc.vector.tensor_copy(
                                    out=V_sb[p][:, sb, :].rearrange(
                                        "q (gg c) -> q gg c", gg=2)[:, :, 0:64],
                                    in_=pv[:, 0:128].rearrange(
                                        "q (gg c) -> q gg c", gg=2))

                        steps += [s1, s2, s3, s4, s5]
                    return steps

                def we_u_steps(u):
                    """Final matmul for output columns [u*256, (u+1)*256)."""
                    steps = []
                    for ob in range(8):
                        def s(ob=ob, u=u):
                            pf = ps512.tile([128, 256], f32, tag="ps512",
                                            name=f"pf{_rep}_{u}_{ob}")
                            for cc in range(4):
                                nc.tensor.matmul(
                                    pf[:],
                                    wet_sb[:, cc, ob * 128:(ob + 1) * 128],
                                    OutMat[:, cc, u * 256:(u + 1) * 256],
                                    start=(cc == 0), stop=(cc == 3))
                            ot = outp.tile([128, 256], f32, tag="outp",
                                           name=f"ot{_rep}_{u}_{ob}")
                            nc.vector.tensor_scalar_add(
                                out=ot[:], in0=pf[:], scalar1=beb_sb[:, ob:ob + 1])
                            nc.sync.dma_start(
                                out[ob * 128:(ob + 1) * 128,
                                    u * 256:(u + 1) * 256], ot[:])
                        steps.append(s)
                    return steps

                # gated filler queue: (gate_t, closure); consumable when the
                # current round index t >= gate_t
                filler = []

                def drain_filler(t, budget=None):
                    n = 0
                    while filler and filler[0][0] <= t and (
                            budget is None or n < budget):
                        filler.pop(0)[1]()
                        n += 1

                # ---------- attention round emitters ----------
                def emit_av_ib(t, ee, ib):
                    rr, p = t // 2, t % 2
                    po = ps128.tile([128, 130], f32, tag="ps128",
                                    name=f"po{_rep}_{t}_{ib}")
                    for grp in range(2):
                        for jc in range(av_jc):
                            sbj = 4 * rr + jc if jc < 4 else 4 * (rr + 4) + (jc - 4)
                            nc.tensor.matmul(
                                po[:, grp * 65:grp * 65 + 65],
                                ee[grp][:, jc, ib * 128:ib * 128 + 128],
                                V_sb[p][:, sbj, grp * 65:grp * 65 + 65],
                                start=(jc == 0), stop=(jc == av_jc - 1))
                    pov = po[:].rearrange("q (gg c) -> q gg c", gg=2)
                    rec = small.tile([128, 2], f32, tag="rec",
                                     name=f"rec{_rep}_{t}_{ib}")
                    nc.vector.reciprocal(out=rec[:], in_=pov[:, :, 64])
                    u = rr if ib < 4 else rr + 4
                    col = u * 256 + p * 128
                    nc.vector.tensor_tensor(
                        OutMat[:, ib % 4, col:col + 128].rearrange(
                            "q (gg c) -> q gg c", gg=2),
                        pov[:, :, 0:64],
                        rec[:, :, None].to_broadcast((128, 2, 64)),
                        mybir.AluOpType.mult)

                def emit_round(t, prev_ee):
                    """E^T + exp for round t, with AV of t-1 and filler woven in."""
                    rr, p = t // 2, t % 2
                    ee = [bigpool.tile([128, 8, 1024], bf16, tag="big",
                                       name=f"ee{_rep}_{t}_{g_}")
                          for g_ in range(2)]
                    for jb in range(8):
                        gj = rr if jb < 4 else rr + 4
                        cj = (jb % 4) * 128
                        pe_t = [pse.tile([128, 1024], f32, tag="pse",
                                         name=f"pe{_rep}_{t}_{jb}_{g_}")
                                for g_ in range(2)]
                        for grp in range(2):
                            rows = slice(grp * 64, grp * 64 + 64)
                            for ic in range(et_ic):
                                gi = rr if ic == 0 else rr + 4
                                nc.tensor.matmul(
                                    pe_t[grp][:, ic * 512:(ic + 1) * 512],
                                    kT[p][rows, gj, cj:cj + 128],
                                    qT[p][rows, gi, :],
                                    start=True, stop=True,
                                    perf_mode=mybir.MatmulPerfMode.DoublePixel,
                                    tile_position=(grp * 64, 0))
                        for grp in range(2):
                            nc.scalar.activation(
                                out=ee[grp][:, jb, :],
                                in_=pe_t[grp][:],
                                func=mybir.ActivationFunctionType.Exp,
                                scale=_SCALE2)
                        if prev_ee is not None:
                            emit_av_ib(t - 1, prev_ee, jb)
                        drain_filler(t, budget=filler_per_jb)
                    return ee

                def pair_steps(gA, gB, x_pre_a=None, x_pre_b=None):
                    """Interleave the two stripes' steps so each q/k dither
                    chain gets unrelated PE work before its ps512 buffer is
                    reused (avoids PE stalls on PSUM pool contention)."""
                    stA = g_block_steps(gA, x_pre=x_pre_a)
                    stB = g_block_steps(gB, x_pre=x_pre_b)
                    return [s for ab in zip(stA, stB) for s in ab]

                # ---------- prologue: stripes for round r=0 ----------
                deferred = []
                st = pair_steps(0, 4, x_pre_a=x_first[0], x_pre_b=x_first[4])
                for i, s in enumerate(st):
                    if i % 10 in (8, 9):  # defer V-blocks (only needed by AV)
                        deferred.append((0, s))
                    else:
                        s()
                nc.sync.dma_start(wet_sb[:], wet[:])

                # queue remaining stripes + We chunks with gates; stripe pairs
                # {1,5}/{2,6}/{3,7} must be fully emitted before rounds t=2/4/6
                # respectively (enforced by the force-drain below); we_u(u) is
                # emittable once AV for its r is fully emitted (AV of round T
                # is woven into round T+1).
                filler.extend(deferred)
                for s in pair_steps(1, 5):
                    filler.append((0, s))
                for s in pair_steps(2, 6):
                    filler.append((1, s))
                for s in pair_steps(3, 7):
                    filler.append((3, s))
                we_gate = {0: 3, 4: 3, 1: 5, 5: 5, 2: 7, 6: 7}
                for u in (0, 4, 1, 5, 2, 6):
                    for s in we_u_steps(u):
                        filler.append((we_gate[u], s))

                prev = None
                for t in range(8):
                    # force-drain stripes required for this round's q/k reads
                    if t in (2, 4, 6):
                        drain_filler({2: 0, 4: 1, 6: 3}[t])
                    prev = emit_round(t, prev)
                drain_filler(7)
                for ib in range(4):
                    emit_av_ib(7, prev, ib)
                for s in we_u_steps(3):
                    s()
                for ib in range(4, 8):
                    emit_av_ib(7, prev, ib)
                for s in we_u_steps(7):
                    s()

    nc.finalize()
    return nc


def _prep_inputs(x, Wq, Wk, Wv, Wr, br, We, be):
    import ml_dtypes

    bfdt = ml_dtypes.bfloat16
    x = np.asarray(x, np.float32)
    Wr = np.asarray(Wr, np.float32)
    We = np.asarray(We, np.float32)

    wrt2 = np.empty((128, 8, 512), np.float32)  # [P, kc, c] = Wr[c, kc*128+P]
    for kc in range(8):
        wrt2[:, kc, :] = Wr[:, kc * 128:(kc + 1) * 128].T
    wet2 = np.empty((128, 4, 1024), np.float32)  # [P, cc, o] = We[o, cc*128+P]
    for cc in range(4):
        wet2[:, cc, :] = We[:, cc * 128:(cc + 1) * 128].T
    brb = np.ascontiguousarray(
        np.broadcast_to(np.asarray(br, np.float32)[None, :], (128, 512)))
    beb = np.ascontiguousarray(np.asarray(be, np.float32).reshape(8, 128).T)

    def bd(w):
        z = np.zeros((128, 128), np.float32)
        wt = np.asarray(w, np.float32).T
        z[:64, :64] = wt
        z[64:, 64:] = wt
        return z

    bdqkv = np.stack([bd(Wq), bd(Wk), bd(Wv)], axis=1)  # [128, 3, 128]
    shared = dict(wrt=wrt2.astype(bfdt), wet=wet2.astype(bfdt), brb=brb,
                  beb=beb, bdqkv=bdqkv.astype(bfdt))
    in_maps = []
    for core in range(NCORES):
        b, hg = core // 2, core % 2
        xv = x[b].reshape(1024, 8, 2, 256)[:, :, hg, :]  # [ch, g, cc]
        xsh = np.empty((8, 128, 8, 256), np.float32)     # [g, P, kc, cc]
        for kc in range(8):
            xsh[:, :, kc, :] = xv[kc * 128:(kc + 1) * 128].transpose(1, 0, 2)
        in_maps.append(dict(xs=xsh.astype(bfdt), **shared))
    return in_maps


def kernel(x, Wq, Wk, Wv, Wr, br, We, be, _trace=False, _pack_e=True):
    from concourse.bass_utils import run_bass_kernel_spmd

    nc = _build_nc(pack_e=_pack_e)
    in_maps = _prep_inputs(x, Wq, Wk, Wv, Wr, br, We, be)
    res = run_bass_kernel_spmd(nc, in_maps, core_ids=list(range(NCORES)),
                               trace=_trace)
    outa = np.zeros((B, EMBED, N), np.float32)
    for core in range(NCORES):
        b, hg = core // 2, core % 2
        oc = res.results[core]["out"]
        outa[b].reshape(1024, 8, 2, 256)[:, :, hg, :] = oc.reshape(1024, 8, 256)
    if _trace:
        kernel._last_results = res
    return outa
